# revision 7
# baseline (speedup 1.0000x reference)
"""Trainium2 Bass kernel for nn_BootstrapRecurrentMixer (B=4, S=2048, D=1024,
INNER=2048, STATE=512).

Strategy: the recurrent scan is strongly contractive (per-sweep error ratio
~0.18, weights scaled 0.02), so instead of a 2048-step serial scan we run a
parallel fixed-point iteration over the whole sequence:

    m_t = g(tok_t, P @ s_t + bp),   s_{t+1} = U @ m_t + bu

iterated 7 times as dense GEMM sweeps (in-place over m, so chunks relax
Gauss-Seidel style).  8 cores split (batch, seq-half); odd cores (second
half) carry a 64-token lead-in window whose boundary state is frozen at
zero — the boundary error attenuates by ~0.2^64, so no cross-core
communication is needed.  Even cores start from the exact given state0.
All GEMMs run as float32r (full PE rate, ~1.5e-4 rounding).

Layouts are feature-major ([feature, token]) end to end; the host pre-
transposes weights/activations when sharding, and the per-partition bias
columns are pre-arranged host-side.
"""
import numpy as np
from contextlib import ExitStack

import concourse.bacc as bacc
import concourse.mybir as mybir
import concourse.tile as tile
import concourse.masks as masks
from concourse.bass_utils import run_bass_kernel_spmd

F32 = mybir.dt.float32
F32R = mybir.dt.float32r
ACTF = mybir.ActivationFunctionType

B, S, D, INNER, STATE = 4, 2048, 1024, 2048, 512
P3 = 3 * INNER    # 6144 in_proj rows
SP2 = 2 * INNER   # 4096 state_proj rows
W = 1056          # token window per core
CH = 352          # chunk (moving free dim; >=256 keeps f32r at full rate)
NCH = W // CH
NITER = 7
WB = W + 1        # m buffer block width (guard column 0)

KD = D // 128      # 8  k-tiles of D
MT3 = P3 // 128    # 48 feature tiles of in_proj output
FT = INNER // 128  # 16 feature tiles of INNER
ST = STATE // 128  # 4  state tiles
KI = INNER // 128  # 16 k-tiles of INNER


def _build():
    nc = bacc.Bacc("TRN2", target_bir_lowering=False, debug=True)

    hT = nc.declare_dram_parameter("hT", [D, W], F32R, isOutput=False)
    WiT = nc.declare_dram_parameter("WiT", [D, P3], F32R, isOutput=False)
    PT = nc.declare_dram_parameter("PT", [STATE, SP2], F32R, isOutput=False)
    UT = nc.declare_dram_parameter("UT", [INNER, STATE], F32R, isOutput=False)
    WoT = nc.declare_dram_parameter("WoT", [INNER, D], F32R, isOutput=False)
    bi_p = nc.declare_dram_parameter("bi_p", [128, MT3], F32, isOutput=False)
    bp_p = nc.declare_dram_parameter("bp_p", [128, 2 * FT], F32, isOutput=False)
    bu_p = nc.declare_dram_parameter("bu_p", [128, ST], F32, isOutput=False)
    bo_p = nc.declare_dram_parameter("bo_p", [128, KD], F32, isOutput=False)
    s_init_p = nc.declare_dram_parameter("s_init", [128, ST], F32, isOutput=False)
    outT = nc.declare_dram_parameter("outT", [D, W], F32, isOutput=True)
    fs = nc.declare_dram_parameter("fs", [128, ST], F32, isOutput=True)

    tok = nc.dram_tensor("tok", [P3, W], F32R)  # in_proj output scratch

    with tile.TileContext(nc) as tc, ExitStack() as ctx:
        # ---- permanent small residents + PT/UT weights ----
        perm = ctx.enter_context(tc.tile_pool(name="perm", bufs=1))
        s_sb = perm.tile([128, ST * W], F32R, tag="s_sb")
        pt_sb = perm.tile([128, ST * SP2], F32R, tag="pt_sb")
        ut_sb = perm.tile([128, KI * STATE], F32R, tag="ut_sb")
        identf = perm.tile([128, 128], F32, tag="identf")
        ident = perm.tile([128, 128], F32R, tag="ident")
        bi_sb = perm.tile([128, MT3], F32, tag="bi_sb")
        bp_sb = perm.tile([128, 2 * FT], F32, tag="bp_sb")
        bu_sb = perm.tile([128, ST], F32, tag="bu_sb")
        bo_sb = perm.tile([128, KD], F32, tag="bo_sb")
        sinit_sb = perm.tile([128, ST], F32, tag="sinit_sb")
        zW = perm.tile([128, W], F32, tag="zW")

        masks.make_identity(nc, identf[:])
        nc.vector.tensor_copy(ident[:], identf[:])
        nc.vector.memset(zW[:], 0.0)
        nc.sync.dma_start(bi_sb[:], bi_p[:])
        nc.sync.dma_start(bp_sb[:], bp_p[:])
        nc.sync.dma_start(bu_sb[:], bu_p[:])
        nc.sync.dma_start(bo_sb[:], bo_p[:])
        nc.sync.dma_start(sinit_sb[:], s_init_p[:])
        for kt in range(ST):
            nc.sync.dma_start(
                pt_sb[:, kt * SP2:(kt + 1) * SP2], PT[kt * 128:(kt + 1) * 128, :]
            )
        for kt in range(KI):
            nc.sync.dma_start(
                ut_sb[:, kt * STATE:(kt + 1) * STATE], UT[kt * 128:(kt + 1) * 128, :]
            )

        # ---- phase A: in_proj -> tok (f32r GEMM, bias/tanh fused at evict) ----
        with tc.tile_pool(name="hT_pool", bufs=1) as hTp, \
             tc.tile_pool(name="wi_pool", bufs=4) as wip, \
             tc.tile_pool(name="psA", bufs=4, space="PSUM") as psA, \
             tc.tile_pool(name="evA", bufs=4) as evA:
            hts = hTp.tile([128, KD * W], F32R, tag="ht")
            nc.sync.dma_start(
                hts[:].rearrange("p (k w) -> p k w", k=KD),
                hT[:].rearrange("(k p) w -> p k w", p=128),
            )
            for mt in range(MT3):
                wi_t = wip.tile([128, KD * 128], F32R, tag="wi")
                nc.sync.dma_start(
                    wi_t[:].rearrange("p (k c) -> p k c", k=KD),
                    WiT[:, mt * 128:(mt + 1) * 128].rearrange("(k p) c -> p k c", p=128),
                )
                ev = evA.tile([128, W], F32R, tag="evA")
                for cn in range(NCH):
                    ps = psA.tile([128, CH], F32, tag="psA")
                    for kt in range(KD):
                        nc.tensor.matmul(
                            ps[:], wi_t[:, kt * 128:(kt + 1) * 128],
                            hts[:, kt * W + cn * CH:kt * W + (cn + 1) * CH],
                            start=(kt == 0), stop=(kt == KD - 1),
                        )
                    evs = ev[:, cn * CH:(cn + 1) * CH]
                    if mt < 2 * FT:  # tu/tg rows: add bias only
                        nc.vector.tensor_scalar_add(evs, ps[:], bi_sb[:, mt:mt + 1])
                    else:            # tv rows: tanh(x + bias)
                        nc.scalar.activation(
                            evs, ps[:], ACTF.Tanh, bias=bi_sb[:, mt:mt + 1]
                        )
                nc.sync.dma_start(tok[mt * 128:(mt + 1) * 128, :], ev[:])

        # ---- m buffer (allocated after phase A pools close to fit SBUF) ----
        perm2 = ctx.enter_context(tc.tile_pool(name="perm2", bufs=1))
        m_sb = perm2.tile([128, KI * WB], F32R, tag="m_sb")
        # only the guard columns (index 0 of each block) must be zero
        for kt in range(KI):
            nc.vector.tensor_copy(m_sb[:, kt * WB:kt * WB + 1], zW[:, 0:1])

        # ---- phase B: fixed-point sweeps ----
        with tc.tile_pool(name="tokp", bufs=8) as tokp, \
             tc.tile_pool(name="psS", bufs=3, space="PSUM") as psS, \
             tc.tile_pool(name="psP", bufs=4, space="PSUM") as psP, \
             tc.tile_pool(name="ewp", bufs=8) as ewp:
            for it in range(NITER):
                for cn in range(NCH):
                    c0 = cn * CH
                    # s-GEMM: s[:, c0:c0+CH] = UT.T @ m[:, tokens c0-1..] + bu
                    # (iteration 0: m == 0, so s == bu directly)
                    for st in range(ST):
                        if it == 0:
                            if cn == 0:
                                nc.vector.tensor_scalar_add(
                                    s_sb[:, st * W:(st + 1) * W], zW[:],
                                    bu_sb[:, st:st + 1],
                                )
                            continue
                        ps = psS.tile([128, CH], F32, tag="psS")
                        for kt in range(KI):
                            nc.tensor.matmul(
                                ps[:],
                                ut_sb[:, kt * STATE + st * 128:kt * STATE + (st + 1) * 128],
                                m_sb[:, kt * WB + c0:kt * WB + c0 + CH],
                                start=(kt == 0), stop=(kt == KI - 1),
                            )
                        nc.vector.tensor_scalar_add(
                            s_sb[:, st * W + c0:st * W + c0 + CH], ps[:],
                            bu_sb[:, st:st + 1],
                        )
                    if cn == 0:
                        for st in range(ST):
                            nc.vector.tensor_copy(
                                s_sb[:, st * W:st * W + 1], sinit_sb[:, st:st + 1]
                            )
                    # SP-GEMM + token fold + elementwise -> m (in place)
                    for ft in range(FT):
                        psu = psP.tile([128, CH], F32, tag="psP")
                        psg = psP.tile([128, CH], F32, tag="psP")
                        for kt in range(ST):
                            nc.tensor.matmul(
                                psu[:],
                                pt_sb[:, kt * SP2 + ft * 128:kt * SP2 + (ft + 1) * 128],
                                s_sb[:, kt * W + c0:kt * W + c0 + CH],
                                start=(kt == 0), stop=False,
                            )
                        tut = tokp.tile([128, CH], F32R, tag="tok")
                        nc.scalar.dma_start(
                            tut[:], tok[ft * 128:(ft + 1) * 128, c0:c0 + CH]
                        )
                        nc.tensor.matmul(
                            psu[:], ident[:], tut[:], start=False, stop=True
                        )
                        for kt in range(ST):
                            nc.tensor.matmul(
                                psg[:],
                                pt_sb[:, kt * SP2 + INNER + ft * 128:kt * SP2 + INNER + (ft + 1) * 128],
                                s_sb[:, kt * W + c0:kt * W + c0 + CH],
                                start=(kt == 0), stop=False,
                            )
                        tgt = tokp.tile([128, CH], F32R, tag="tok")
                        nc.scalar.dma_start(
                            tgt[:], tok[(FT + ft) * 128:(FT + ft + 1) * 128, c0:c0 + CH]
                        )
                        nc.tensor.matmul(
                            psg[:], ident[:], tgt[:], start=False, stop=True
                        )
                        cand = ewp.tile([128, CH], F32, tag="ew")
                        nc.scalar.activation(
                            cand[:], psu[:], ACTF.Tanh, bias=bp_sb[:, ft:ft + 1]
                        )
                        gate = ewp.tile([128, CH], F32, tag="ew")
                        nc.scalar.activation(
                            gate[:], psg[:], ACTF.Sigmoid, bias=bp_sb[:, FT + ft:FT + ft + 1]
                        )
                        tvtt = tokp.tile([128, CH], F32R, tag="tok")
                        nc.sync.dma_start(
                            tvtt[:], tok[(2 * FT + ft) * 128:(2 * FT + ft + 1) * 128, c0:c0 + CH]
                        )
                        dd = ewp.tile([128, CH], F32, tag="ew")
                        nc.vector.tensor_sub(dd[:], cand[:], tvtt[:])
                        ee = ewp.tile([128, CH], F32, tag="ew")
                        nc.vector.tensor_mul(ee[:], gate[:], dd[:])
                        nc.vector.tensor_add(
                            m_sb[:, ft * WB + c0 + 1:ft * WB + c0 + 1 + CH],
                            ee[:], tvtt[:],
                        )

        # ---- phase C: out_proj + final state ----
        with tc.tile_pool(name="wo_pool", bufs=2) as wop, \
             tc.tile_pool(name="psC", bufs=4, space="PSUM") as psC, \
             tc.tile_pool(name="evC", bufs=3) as evC:
            for dt_ in range(KD):
                wo_t = wop.tile([128, KI * 128], F32R, tag="wo")
                nc.sync.dma_start(
                    wo_t[:].rearrange("p (k c) -> p k c", k=KI),
                    WoT[:, dt_ * 128:(dt_ + 1) * 128].rearrange("(k p) c -> p k c", p=128),
                )
                for cn in range(NCH):
                    c0 = cn * CH
                    ps = psC.tile([128, CH], F32, tag="psC")
                    for kt in range(KI):
                        nc.tensor.matmul(
                            ps[:], wo_t[:, kt * 128:(kt + 1) * 128],
                            m_sb[:, kt * WB + c0 + 1:kt * WB + c0 + 1 + CH],
                            start=(kt == 0), stop=(kt == KI - 1),
                        )
                    ev = evC.tile([128, CH], F32, tag="evC")
                    nc.vector.tensor_scalar_add(ev[:], ps[:], bo_sb[:, dt_:dt_ + 1])
                    nc.sync.dma_start(
                        outT[dt_ * 128:(dt_ + 1) * 128, c0:c0 + CH], ev[:]
                    )
            # final state: s(token W) = UT.T @ m[:, token W-1] + bu
            fs_sb = evC.tile([128, ST], F32, tag="fs_sb")
            for st in range(ST):
                ps = psC.tile([128, 8], F32, tag="psFS")
                for kt in range(KI):
                    nc.tensor.matmul(
                        ps[:],
                        ut_sb[:, kt * STATE + st * 128:kt * STATE + (st + 1) * 128],
                        m_sb[:, kt * WB + W - 7:kt * WB + W + 1],
                        start=(kt == 0), stop=(kt == KI - 1),
                    )
                nc.vector.tensor_scalar_add(
                    fs_sb[:, st:st + 1], ps[:, 7:8], bu_sb[:, st:st + 1]
                )
            nc.sync.dma_start(fs[:], fs_sb[:])

    nc.finalize()
    return nc


_program = None


def kernel(**inputs):
    global _program
    hidden = np.ascontiguousarray(np.asarray(inputs["hidden"], np.float32))
    state0 = np.asarray(inputs["state0"], np.float32)
    WiT = np.ascontiguousarray(np.asarray(inputs["in_proj_w"], np.float32).T)
    bi = np.asarray(inputs["in_proj_b"], np.float32)
    PT = np.ascontiguousarray(np.asarray(inputs["state_proj_w"], np.float32).T)
    bp = np.asarray(inputs["state_proj_b"], np.float32)
    WoT = np.ascontiguousarray(np.asarray(inputs["out_proj_w"], np.float32).T)
    bo = np.asarray(inputs["out_proj_b"], np.float32)
    UT = np.ascontiguousarray(np.asarray(inputs["state_update_w"], np.float32).T)
    bu = np.asarray(inputs["state_update_b"], np.float32)

    bi_p = np.ascontiguousarray(bi.reshape(MT3, 128).T)
    bp_p = np.ascontiguousarray(bp.reshape(2 * FT, 128).T)
    bu_p = np.ascontiguousarray(bu.reshape(ST, 128).T)
    bo_p = np.ascontiguousarray(bo.reshape(KD, 128).T)

    in_maps = []
    for c in range(8):
        b, half = c // 2, c % 2
        s0 = 0 if half == 0 else S - W
        hT_c = np.ascontiguousarray(hidden[b, s0:s0 + W, :].T)
        if half == 0:
            s_init = np.ascontiguousarray(state0[b].reshape(ST, 128).T)
        else:
            s_init = np.zeros((128, ST), np.float32)
        in_maps.append({
            "hT": hT_c, "WiT": WiT, "PT": PT, "UT": UT, "WoT": WoT,
            "bi_p": bi_p, "bp_p": bp_p, "bu_p": bu_p, "bo_p": bo_p,
            "s_init": s_init,
        })

    if _program is None:
        _program = _build()
    res = run_bass_kernel_spmd(_program, in_maps, list(range(8)))

    out = np.empty((B, S, D), np.float32)
    final_state = np.empty((B, STATE), np.float32)
    for b in range(B):
        outT_e = res.results[2 * b]["outT"]       # [D, W] tokens 0..1056
        outT_o = res.results[2 * b + 1]["outT"]   # [D, W] tokens 992..2048
        out[b, :W] = outT_e.T
        out[b, W:] = outT_o.T[W - (S - W):]
        final_state[b] = res.results[2 * b + 1]["fs"].T.reshape(STATE)
    return out, final_state


# revision 8
# speedup vs baseline: 1.1309x; 1.1309x over previous
"""Trainium2 Bass kernel for nn_BootstrapRecurrentMixer (B=4, S=2048, D=1024,
INNER=2048, STATE=512).

Strategy: the recurrent scan is strongly contractive (per-sweep error ratio
~0.18, weights scaled 0.02), so instead of a 2048-step serial scan we run a
parallel fixed-point iteration over the whole sequence:

    m_t = g(tok_t, P @ s_t + bp),   s_{t+1} = U @ m_t + bu

iterated 7 times as dense GEMM sweeps (in-place over m, so chunks relax
Gauss-Seidel style).  8 cores split (batch, seq-half); odd cores (second
half) carry a 64-token lead-in window whose boundary state is frozen at
zero — the boundary error attenuates by ~0.2^64, so no cross-core
communication is needed.  Even cores start from the exact given state0.
All GEMMs run as float32r (full PE rate, ~1.5e-4 rounding).

Layouts are feature-major ([feature, token]) end to end; the host pre-
transposes weights/activations when sharding, and the per-partition bias
columns are pre-arranged host-side.
"""
import numpy as np
from contextlib import ExitStack

import concourse.bacc as bacc
import concourse.mybir as mybir
import concourse.tile as tile
import concourse.masks as masks
from concourse.bass_utils import run_bass_kernel_spmd

F32 = mybir.dt.float32
F32R = mybir.dt.float32r
ACTF = mybir.ActivationFunctionType

B, S, D, INNER, STATE = 4, 2048, 1024, 2048, 512
P3 = 3 * INNER    # 6144 in_proj rows
SP2 = 2 * INNER   # 4096 state_proj rows
W = 1056          # token window per core
CH = 352          # chunk (moving free dim; >=256 keeps f32r at full rate)
NCH = W // CH
NITER = 6
WB = W + 1        # m buffer block width (guard column 0)

KD = D // 128      # 8  k-tiles of D
MT3 = P3 // 128    # 48 feature tiles of in_proj output
FT = INNER // 128  # 16 feature tiles of INNER
ST = STATE // 128  # 4  state tiles
KI = INNER // 128  # 16 k-tiles of INNER


def _build():
    nc = bacc.Bacc("TRN2", target_bir_lowering=False, debug=True)

    hT = nc.declare_dram_parameter("hT", [D, W], F32R, isOutput=False)
    WiT = nc.declare_dram_parameter("WiT", [D, P3], F32R, isOutput=False)
    PT = nc.declare_dram_parameter("PT", [STATE, SP2], F32R, isOutput=False)
    UT = nc.declare_dram_parameter("UT", [INNER, STATE], F32R, isOutput=False)
    WoT = nc.declare_dram_parameter("WoT", [INNER, D], F32R, isOutput=False)
    bi_p = nc.declare_dram_parameter("bi_p", [128, MT3], F32, isOutput=False)
    bp_p = nc.declare_dram_parameter("bp_p", [128, 2 * FT], F32, isOutput=False)
    bu_p = nc.declare_dram_parameter("bu_p", [128, ST], F32, isOutput=False)
    bo_p = nc.declare_dram_parameter("bo_p", [128, KD], F32, isOutput=False)
    s_init_p = nc.declare_dram_parameter("s_init", [128, ST], F32, isOutput=False)
    outT = nc.declare_dram_parameter("outT", [D, W], F32, isOutput=True)
    fs = nc.declare_dram_parameter("fs", [128, ST], F32, isOutput=True)

    tok = nc.dram_tensor("tok", [P3, W], F32R)  # in_proj output scratch

    with tile.TileContext(nc) as tc, ExitStack() as ctx:
        # ---- permanent small residents + PT/UT weights ----
        perm = ctx.enter_context(tc.tile_pool(name="perm", bufs=1))
        s_sb = perm.tile([128, ST * W], F32R, tag="s_sb")
        pt_sb = perm.tile([128, ST * SP2], F32R, tag="pt_sb")
        ut_sb = perm.tile([128, KI * STATE], F32R, tag="ut_sb")
        identf = perm.tile([128, 128], F32, tag="identf")
        ident = perm.tile([128, 128], F32R, tag="ident")
        bi_sb = perm.tile([128, MT3], F32, tag="bi_sb")
        bp_sb = perm.tile([128, 2 * FT], F32, tag="bp_sb")
        bu_sb = perm.tile([128, ST], F32, tag="bu_sb")
        bo_sb = perm.tile([128, KD], F32, tag="bo_sb")
        sinit_sb = perm.tile([128, ST], F32, tag="sinit_sb")
        zW = perm.tile([128, W], F32, tag="zW")

        masks.make_identity(nc, identf[:])
        nc.vector.tensor_copy(ident[:], identf[:])
        nc.vector.memset(zW[:], 0.0)
        nc.sync.dma_start(bi_sb[:], bi_p[:])
        nc.sync.dma_start(bp_sb[:], bp_p[:])
        nc.sync.dma_start(bu_sb[:], bu_p[:])
        nc.sync.dma_start(bo_sb[:], bo_p[:])
        nc.sync.dma_start(sinit_sb[:], s_init_p[:])
        for kt in range(ST):
            nc.sync.dma_start(
                pt_sb[:, kt * SP2:(kt + 1) * SP2], PT[kt * 128:(kt + 1) * 128, :]
            )
        for kt in range(KI):
            nc.sync.dma_start(
                ut_sb[:, kt * STATE:(kt + 1) * STATE], UT[kt * 128:(kt + 1) * 128, :]
            )

        # ---- phase A: in_proj -> tok (f32r GEMM, bias/tanh fused at evict) ----
        with tc.tile_pool(name="hT_pool", bufs=1) as hTp, \
             tc.tile_pool(name="wi_pool", bufs=4) as wip, \
             tc.tile_pool(name="psA", bufs=4, space="PSUM") as psA, \
             tc.tile_pool(name="evA", bufs=4) as evA:
            hts = hTp.tile([128, KD * W], F32R, tag="ht")
            nc.sync.dma_start(
                hts[:].rearrange("p (k w) -> p k w", k=KD),
                hT[:].rearrange("(k p) w -> p k w", p=128),
            )
            for mt in range(MT3):
                wi_t = wip.tile([128, KD * 128], F32R, tag="wi")
                nc.sync.dma_start(
                    wi_t[:].rearrange("p (k c) -> p k c", k=KD),
                    WiT[:, mt * 128:(mt + 1) * 128].rearrange("(k p) c -> p k c", p=128),
                )
                ev = evA.tile([128, W], F32R, tag="evA")
                for cn in range(NCH):
                    ps = psA.tile([128, CH], F32, tag="psA")
                    for kt in range(KD):
                        nc.tensor.matmul(
                            ps[:], wi_t[:, kt * 128:(kt + 1) * 128],
                            hts[:, kt * W + cn * CH:kt * W + (cn + 1) * CH],
                            start=(kt == 0), stop=(kt == KD - 1),
                        )
                    evs = ev[:, cn * CH:(cn + 1) * CH]
                    if mt < 2 * FT:  # tu/tg rows: add bias only
                        nc.vector.tensor_scalar_add(evs, ps[:], bi_sb[:, mt:mt + 1])
                    else:            # tv rows: tanh(x + bias)
                        nc.scalar.activation(
                            evs, ps[:], ACTF.Tanh, bias=bi_sb[:, mt:mt + 1]
                        )
                nc.sync.dma_start(tok[mt * 128:(mt + 1) * 128, :], ev[:])

        # ---- m buffer (allocated after phase A pools close to fit SBUF) ----
        perm2 = ctx.enter_context(tc.tile_pool(name="perm2", bufs=1))
        m_sb = perm2.tile([128, KI * WB], F32R, tag="m_sb")
        # only the guard columns (index 0 of each block) must be zero
        for kt in range(KI):
            nc.vector.tensor_copy(m_sb[:, kt * WB:kt * WB + 1], zW[:, 0:1])

        # ---- phase B: fixed-point sweeps ----
        with tc.tile_pool(name="tokp", bufs=8) as tokp, \
             tc.tile_pool(name="psS", bufs=3, space="PSUM") as psS, \
             tc.tile_pool(name="psP", bufs=4, space="PSUM") as psP, \
             tc.tile_pool(name="ewp", bufs=8) as ewp:
            for it in range(NITER):
                for cn in range(NCH):
                    c0 = cn * CH
                    # s-GEMM: s[:, c0:c0+CH] = UT.T @ m[:, tokens c0-1..] + bu
                    # (iteration 0: m == 0, so s == bu directly)
                    for st in range(ST):
                        if it == 0:
                            if cn == 0:
                                nc.vector.tensor_scalar_add(
                                    s_sb[:, st * W:(st + 1) * W], zW[:],
                                    bu_sb[:, st:st + 1],
                                )
                            continue
                        ps = psS.tile([128, CH], F32, tag="psS")
                        for kt in range(KI):
                            nc.tensor.matmul(
                                ps[:],
                                ut_sb[:, kt * STATE + st * 128:kt * STATE + (st + 1) * 128],
                                m_sb[:, kt * WB + c0:kt * WB + c0 + CH],
                                start=(kt == 0), stop=(kt == KI - 1),
                            )
                        nc.vector.tensor_scalar_add(
                            s_sb[:, st * W + c0:st * W + c0 + CH], ps[:],
                            bu_sb[:, st:st + 1],
                        )
                    if cn == 0:
                        for st in range(ST):
                            nc.vector.tensor_copy(
                                s_sb[:, st * W:st * W + 1], sinit_sb[:, st:st + 1]
                            )
                    # SP-GEMM + token fold + elementwise -> m (in place)
                    for ft in range(FT):
                        psu = psP.tile([128, CH], F32, tag="psP")
                        psg = psP.tile([128, CH], F32, tag="psP")
                        for kt in range(ST):
                            nc.tensor.matmul(
                                psu[:],
                                pt_sb[:, kt * SP2 + ft * 128:kt * SP2 + (ft + 1) * 128],
                                s_sb[:, kt * W + c0:kt * W + c0 + CH],
                                start=(kt == 0), stop=False,
                            )
                        tut = tokp.tile([128, CH], F32R, tag="tok")
                        nc.scalar.dma_start(
                            tut[:], tok[ft * 128:(ft + 1) * 128, c0:c0 + CH]
                        )
                        nc.tensor.matmul(
                            psu[:], ident[:], tut[:], start=False, stop=True
                        )
                        for kt in range(ST):
                            nc.tensor.matmul(
                                psg[:],
                                pt_sb[:, kt * SP2 + INNER + ft * 128:kt * SP2 + INNER + (ft + 1) * 128],
                                s_sb[:, kt * W + c0:kt * W + c0 + CH],
                                start=(kt == 0), stop=False,
                            )
                        tgt = tokp.tile([128, CH], F32R, tag="tok")
                        nc.scalar.dma_start(
                            tgt[:], tok[(FT + ft) * 128:(FT + ft + 1) * 128, c0:c0 + CH]
                        )
                        nc.tensor.matmul(
                            psg[:], ident[:], tgt[:], start=False, stop=True
                        )
                        cand = ewp.tile([128, CH], F32, tag="ew")
                        nc.scalar.activation(
                            cand[:], psu[:], ACTF.Tanh, bias=bp_sb[:, ft:ft + 1]
                        )
                        gate = ewp.tile([128, CH], F32, tag="ew")
                        nc.scalar.activation(
                            gate[:], psg[:], ACTF.Sigmoid, bias=bp_sb[:, FT + ft:FT + ft + 1]
                        )
                        tvtt = tokp.tile([128, CH], F32R, tag="tok")
                        nc.sync.dma_start(
                            tvtt[:], tok[(2 * FT + ft) * 128:(2 * FT + ft + 1) * 128, c0:c0 + CH]
                        )
                        dd = ewp.tile([128, CH], F32, tag="ew")
                        nc.vector.tensor_sub(dd[:], cand[:], tvtt[:])
                        ee = ewp.tile([128, CH], F32, tag="ew")
                        nc.vector.tensor_mul(ee[:], gate[:], dd[:])
                        nc.vector.tensor_add(
                            m_sb[:, ft * WB + c0 + 1:ft * WB + c0 + 1 + CH],
                            ee[:], tvtt[:],
                        )

        # ---- phase C: out_proj + final state ----
        with tc.tile_pool(name="wo_pool", bufs=2) as wop, \
             tc.tile_pool(name="psC", bufs=4, space="PSUM") as psC, \
             tc.tile_pool(name="evC", bufs=3) as evC:
            for dt_ in range(KD):
                wo_t = wop.tile([128, KI * 128], F32R, tag="wo")
                nc.sync.dma_start(
                    wo_t[:].rearrange("p (k c) -> p k c", k=KI),
                    WoT[:, dt_ * 128:(dt_ + 1) * 128].rearrange("(k p) c -> p k c", p=128),
                )
                for cn in range(NCH):
                    c0 = cn * CH
                    ps = psC.tile([128, CH], F32, tag="psC")
                    for kt in range(KI):
                        nc.tensor.matmul(
                            ps[:], wo_t[:, kt * 128:(kt + 1) * 128],
                            m_sb[:, kt * WB + c0 + 1:kt * WB + c0 + 1 + CH],
                            start=(kt == 0), stop=(kt == KI - 1),
                        )
                    ev = evC.tile([128, CH], F32, tag="evC")
                    nc.vector.tensor_scalar_add(ev[:], ps[:], bo_sb[:, dt_:dt_ + 1])
                    nc.sync.dma_start(
                        outT[dt_ * 128:(dt_ + 1) * 128, c0:c0 + CH], ev[:]
                    )
            # final state: s(token W) = UT.T @ m[:, token W-1] + bu
            fs_sb = evC.tile([128, ST], F32, tag="fs_sb")
            for st in range(ST):
                ps = psC.tile([128, 8], F32, tag="psFS")
                for kt in range(KI):
                    nc.tensor.matmul(
                        ps[:],
                        ut_sb[:, kt * STATE + st * 128:kt * STATE + (st + 1) * 128],
                        m_sb[:, kt * WB + W - 7:kt * WB + W + 1],
                        start=(kt == 0), stop=(kt == KI - 1),
                    )
                nc.vector.tensor_scalar_add(
                    fs_sb[:, st:st + 1], ps[:, 7:8], bu_sb[:, st:st + 1]
                )
            nc.sync.dma_start(fs[:], fs_sb[:])

    nc.finalize()
    return nc


_program = None


def kernel(**inputs):
    global _program
    hidden = np.ascontiguousarray(np.asarray(inputs["hidden"], np.float32))
    state0 = np.asarray(inputs["state0"], np.float32)
    WiT = np.ascontiguousarray(np.asarray(inputs["in_proj_w"], np.float32).T)
    bi = np.asarray(inputs["in_proj_b"], np.float32)
    PT = np.ascontiguousarray(np.asarray(inputs["state_proj_w"], np.float32).T)
    bp = np.asarray(inputs["state_proj_b"], np.float32)
    WoT = np.ascontiguousarray(np.asarray(inputs["out_proj_w"], np.float32).T)
    bo = np.asarray(inputs["out_proj_b"], np.float32)
    UT = np.ascontiguousarray(np.asarray(inputs["state_update_w"], np.float32).T)
    bu = np.asarray(inputs["state_update_b"], np.float32)

    bi_p = np.ascontiguousarray(bi.reshape(MT3, 128).T)
    bp_p = np.ascontiguousarray(bp.reshape(2 * FT, 128).T)
    bu_p = np.ascontiguousarray(bu.reshape(ST, 128).T)
    bo_p = np.ascontiguousarray(bo.reshape(KD, 128).T)

    in_maps = []
    for c in range(8):
        b, half = c // 2, c % 2
        s0 = 0 if half == 0 else S - W
        hT_c = np.ascontiguousarray(hidden[b, s0:s0 + W, :].T)
        if half == 0:
            s_init = np.ascontiguousarray(state0[b].reshape(ST, 128).T)
        else:
            s_init = np.zeros((128, ST), np.float32)
        in_maps.append({
            "hT": hT_c, "WiT": WiT, "PT": PT, "UT": UT, "WoT": WoT,
            "bi_p": bi_p, "bp_p": bp_p, "bu_p": bu_p, "bo_p": bo_p,
            "s_init": s_init,
        })

    if _program is None:
        _program = _build()
    res = run_bass_kernel_spmd(_program, in_maps, list(range(8)))

    out = np.empty((B, S, D), np.float32)
    final_state = np.empty((B, STATE), np.float32)
    for b in range(B):
        outT_e = res.results[2 * b]["outT"]       # [D, W] tokens 0..1056
        outT_o = res.results[2 * b + 1]["outT"]   # [D, W] tokens 992..2048
        out[b, :W] = outT_e.T
        out[b, W:] = outT_o.T[W - (S - W):]
        final_state[b] = res.results[2 * b + 1]["fs"].T.reshape(STATE)
    return out, final_state
